# revision 23
# baseline (speedup 1.0000x reference)
"""Trainium2 Bass kernel for AdDiffSortLoss (v2).

Reference computation (per batch row, n=8):
  rank_r      = # { j : labels[j] > labels[r] }          (descending rank)
  G[r, c]     = (rank_r == c)                            (one-hot GT permutation^T)
  x           = -(pred - rank_ema[rank])                 (rank_ema == 0 in practice)
  P           = odd-even differentiable sort network on x (8 layers, Cauchy CDF)
  loss        = -mean( G*clip(log P,-100) + (1-G)*clip(log1p(-P),-100) )

Loss decomposition on device (clips never bind: P in [1.9e-10, 0.975]):
  sum = SUM_all ln(1-P) + SUM_r [ ln(P[r,rank_r]) - ln(1-P[r,rank_r]) ]
  loss = -sum / (B*64)

Key implementation facts (hardware-verified this session):
  * The ACT Arctan table is accurate over the FULL input range (err <= 3e-7
    f32-out; f16-out limited only by f16 rounding), so alpha =
    0.5 + arctan(d)/pi needs NO range reduction: one table lookup per
    comparator, with the affine (1/pi, +0.5) folded into a second ACT
    Identity op (scale+bias). This deletes the |d|/reciprocal/is_ge/sign
    pipeline of the previous version.
  * On real HW, tensor_tensor with 2-byte dtypes and stride-1 innermost APs
    runs at ~0.475 ns/elem (2x); InstTensorScalarPtr runs at 1x (~1.1)
    DESPITE the cost model advertising 4x for it. All bulk math is therefore
    tensor_tensor.
  * Values run in f16 (|x| up to ~50 needs ~1e-3 abs resolution; bf16 is
    too coarse), the permutation mixing in bf16 (P entries down to 1.9e-10
    underflow f16). Ranks from bf16 labels (flips average out). End-to-end
    rel-err vs the f64 reference: 3.2e-04 (gate: 2e-2).
  * Layers 0+1 of the mixing are constructed ANALYTICALLY from the layer-0/1
    alphas (every P2 entry is a 2-term alpha product), which also removes
    the full ping-pong buffer zero-fill: only the 36 flat positions that
    later layers read-before-write are zeroed (small Pool memsets).
    Later layers write only their true column supports (tight windows).
  * Rank is computed with 7 shifted stride-1 compares + ragged in-place
    accumulates instead of an 8x8 broadcast compare matrix (broadcast
    operands break the 2x DVE mode).
  * The BCE ln scratch must NOT alias the idle ping-pong buffer: that WAW
    chained each chunk's ACT work into the next chunk's buffer reuse and
    cost ~80us/pass in stalls.

Engine split per pass: DVE does deltas/value updates/mixing/rank/pick;
ACT does arctan + alpha assembly + dtype casts + the P2 ingredient builds
and all BCE logs (with accumulate); Pool (GPSIMD) does the margin zeros.
Phase 2 runs in 64-row chunks with double-buffered PT tiles so chunk k+1
overlaps chunk k.

Sharding: pure data parallel over the batch across 8 NeuronCores; each core
reduces its shard to a [128,1] per-partition partial that the host sums.
"""

import math
import numpy as np

import concourse.bass as bass
import concourse.bacc as bacc
import concourse.tile as tile
from concourse import mybir
from concourse.bass_utils import run_bass_kernel_spmd

import ml_dtypes

F32 = mybir.dt.float32
F16 = mybir.dt.float16
BF16 = mybir.dt.bfloat16

N = 8                  # row width
N_CORES = 8
BATCH = 262144
CHUNK_ROWS = 64
ROWS_PER_CORE = BATCH // N_CORES   # 32768
P = 128                # partitions
RPP = ROWS_PER_CORE // P           # rows per partition (256)

A = mybir.AluOpType
AF = mybir.ActivationFunctionType
INV_PI = 1.0 / math.pi


def build_nc(rows_per_core=ROWS_PER_CORE, chunk_rows=64, mix_bf16=True,
             repeats=1, parts="all"):
    """Build the single-core SPMD Bass graph.

    chunk_rows: rows-per-partition per phase-2 chunk (must divide rows/128).
    repeats: process the whole shard this many times (timing builds only).
    """
    rpp = rows_per_core // P
    assert rpp * P == rows_per_core
    F = chunk_rows
    n_chunks = rpp // F
    assert n_chunks * F == rpp

    nc = bacc.Bacc("TRN2")

    pred_h = nc.declare_dram_parameter("pred", [rows_per_core, N], F32, isOutput=False)
    lab_h = nc.declare_dram_parameter("labels", [rows_per_core, N], F32, isOutput=False)
    # iota_cr[p, c*8+r] = c (replicated across partitions) -- for GT construction
    iota_h = nc.declare_dram_parameter("iota_cr", [P, N * N + N], BF16, isOutput=False)
    out_h = nc.declare_dram_parameter("out", [P, 1], F32, isOutput=True)

    predv = pred_h[:].rearrange("(p f) n -> p f n", p=P)   # [128, rpp, 8]
    labv = lab_h[:].rearrange("(p f) n -> p f n", p=P)

    def tt(out, in0, in1, op, engine=None):
        (engine or nc.vector).tensor_tensor(out, in0, in1, op)

    with tile.TileContext(nc) as tc:
        with (
            tc.tile_pool(name="io", bufs=2) as io,
            tc.tile_pool(name="rk", bufs=1) as rk,
            tc.tile_pool(name="pp", bufs=2 if chunk_rows <= 64 else 1) as pp,
            tc.tile_pool(name="vt", bufs=1) as vt,
            tc.tile_pool(name="als", bufs=1) as als,
            tc.tile_pool(name="mt", bufs=1) as mt,
            tc.tile_pool(name="acc", bufs=4) as accp,
            tc.tile_pool(name="singles", bufs=1) as singles,
        ):
            # constants
            iota_t = singles.tile([P, N * N + N], BF16, tag="iota")
            nc.sync.dma_start(out=iota_t, in_=iota_h[:])
            # iota_r[p, r] = r lives in the constant's 8-wide tail
            iota_r = iota_t[:, N * N:N * N + N]
            half_t = singles.tile([P, 1], F32, tag="half")
            nc.vector.memset(half_t, 0.5)
            one_t = singles.tile([P, 1], F32, tag="one")
            nc.vector.memset(one_t, 1.0)
            total_t = singles.tile([P, 1], F32, tag="total")
            nc.vector.tensor_scalar(
                total_t, iota_t[:, 0:1], 0.0, None, A.mult
            )

            for _ in range(repeats):
                # ---- input loads: 4 DMA queues per tensor ----------------
                pred_t = io.tile([P, rpp, N], F32, tag="pred")
                lab_t = io.tile([P, rpp, N], F32, tag="lab")
                q = rpp // 4
                for i in range(4):
                    nc.sync.dma_start(out=pred_t[:, i * q:(i + 1) * q, :],
                                      in_=predv[:, i * q:(i + 1) * q, :])
                    nc.sync.dma_start(out=lab_t[:, i * q:(i + 1) * q, :],
                                      in_=labv[:, i * q:(i + 1) * q, :])

                # ---- casts on ACT ----------------------------------------
                # x slot-major [P, slot, f] f16; x = -10*pred
                x_a = vt.tile([P, N, rpp], F16, tag="x_a")
                x_b = vt.tile([P, N, rpp], F16, tag="x_b")
                nc.scalar.activation(
                    x_a,
                    bass.AP(tensor=pred_t.tensor, offset=pred_t.offset,
                            ap=[pred_t.ap[0], [1, N], [N, rpp]]),
                    AF.Identity, scale=-10.0,
                )
                labq = rk.tile([P, rpp, N], BF16, tag="labq")
                nc.scalar.activation(labq, lab_t, AF.Identity)

                # ---- rank via shifted comparisons (full width, bf16) -----
                rank_t = rk.tile([P, rpp, N], BF16, tag="rank")
                nc.vector.tensor_copy(
                    rank_t,
                    bass.AP(tensor=iota_r.tensor, offset=iota_r.offset,
                            ap=[iota_r.ap[0], [0, rpp], [1, N]]),
                )
                cs = rk.tile([P, rpp, N - 1], BF16, tag="cs")

                def rank_step(s):
                    # rank starts at r; j>r adds c_s, j<r adds 1-c_s whose +1
                    # is pre-folded into the iota init, leaving -c_s.
                    w = N - s
                    c_s = cs[:, :, 0:w]
                    tt(c_s, labq[:, :, s:N], labq[:, :, 0:N - s], A.is_gt)
                    tt(rank_t[:, :, 0:w], rank_t[:, :, 0:w], c_s, A.add)
                    tt(rank_t[:, :, s:N], rank_t[:, :, s:N], c_s, A.subtract)

                # ---- phase 1: value recurrence + alphas ------------------
                al2s = []
                alfs = []
                x_cur, x_nxt = x_a, x_b
                for layer in range(N):
                    st = layer % 2
                    npair = (N - st) // 2
                    # slot-major slices: [pair, f], innermost f stride 1
                    def slot_ap(x, base):
                        return bass.AP(
                            tensor=x.tensor, offset=x.offset + base * rpp,
                            ap=[x.ap[0], [2 * rpp, npair], [1, rpp]],
                        )
                    a_ap = slot_ap(x_cur, st)
                    b_ap = slot_ap(x_cur, st + 1)

                    delta = vt.tile([P, npair, rpp], F16, tag="delta")
                    tt(delta, b_ap, a_ap, A.subtract)
                    at16 = vt.tile([P, npair, rpp], F16, tag="at16")
                    nc.scalar.activation(at16, delta, AF.Arctan)
                    # alpha = at/pi + 0.5 assembled ON ACT (scale+bias).
                    # Only this op is on the recurrence critical path; the
                    # mixing-alpha dups and 1-alpha ops are deferred below
                    # so the in-order ACT queue stays on the recurrence.
                    alf = vt.tile([P, npair, rpp], F16, tag=f"alf_{layer}")
                    nc.scalar.activation(alf, at16, AF.Identity,
                                         scale=INV_PI, bias=half_t)
                    alfs.append(alf)
                    if layer < N - 1:
                        # fill the DVE bubble (waiting on ACT) with one
                        # independent rank step
                        rank_step(layer + 1)

                    if layer < N - 1:
                        tv = vt.tile([P, npair, rpp], F16, tag="tv")
                        tt(tv, alf, delta, A.mult)
                        tt(slot_ap(x_nxt, st), b_ap, tv, A.subtract)
                        tt(slot_ap(x_nxt, st + 1), a_ap, tv, A.add)
                        if st == 1:  # passthrough slots 0 and 7
                            nc.vector.tensor_copy(
                                bass.AP(tensor=x_nxt.tensor, offset=x_nxt.offset,
                                        ap=[x_nxt.ap[0], [(N - 1) * rpp, 2], [1, rpp]]),
                                bass.AP(tensor=x_cur.tensor, offset=x_cur.offset,
                                        ap=[x_cur.ap[0], [(N - 1) * rpp, 2], [1, rpp]]),
                            )
                        x_cur, x_nxt = x_nxt, x_cur

                # ---- deferred ACT work (off the recurrence path) ---------
                alfs01 = []
                for layer in (0, 1):
                    npair_l = (N - layer % 2) // 2
                    alm = vt.tile([P, npair_l, rpp], F16, tag=f"alm_{layer}")
                    nc.scalar.activation(alm, alfs[layer], AF.Identity,
                                         scale=-1.0, bias=one_t)
                    alfs01.append((alfs[layer], alm))
                    al2s.append(None)
                for layer in range(2, N):
                    npair_l = (N - layer % 2) // 2
                    alf = alfs[layer]
                    al2 = als.tile([P, rpp, npair_l, 2], BF16,
                                   tag=f"al2_{layer}")
                    nc.scalar.activation(
                        al2,
                        bass.AP(tensor=alf.tensor, offset=alf.offset,
                                ap=[alf.ap[0], [1, rpp], [rpp, npair_l],
                                    [0, 2]]),
                        AF.Identity,
                    )
                    al2s.append(al2)

                # ---- analytic P2 ingredients (bf16, full width) ----------
                # A3[f,k,:] = [1-a0_k, a0_k, a0_{k+1}, 1-a0_{k+1}]
                # B6[f,k,:] = [b_k, b_k, 1-b_k, 1-b_k, b_k, b_k]
                # E4[f,:]   = [a0_0, 1-a0_0, 1-a0_3, a0_3]
                (alf0, alm0), (alf1, alm1) = alfs01
                A3 = rk.tile([P, rpp, 3, 4], BF16, tag="A3")
                for j, (src, p0) in enumerate([(alm0, 0), (alf0, 0),
                                               (alf0, 1), (alm0, 1)]):
                    nc.scalar.activation(
                        bass.AP(tensor=A3.tensor, offset=A3.offset + j,
                                ap=[A3.ap[0], [12, rpp], [4, 3]]),
                        bass.AP(tensor=src.tensor, offset=src.offset + p0 * rpp,
                                ap=[src.ap[0], [1, rpp], [rpp, 3]]),
                        AF.Identity,
                    )
                B6 = rk.tile([P, rpp, 3, 6], BF16, tag="B6")
                for j0, src in [(0, alf1), (2, alm1), (4, alf1)]:
                    nc.scalar.activation(
                        bass.AP(tensor=B6.tensor, offset=B6.offset + j0,
                                ap=[B6.ap[0], [18, rpp], [6, 3], [1, 2]]),
                        bass.AP(tensor=src.tensor, offset=src.offset,
                                ap=[src.ap[0], [1, rpp], [rpp, 3], [0, 2]]),
                        AF.Identity,
                    )
                E4 = rk.tile([P, rpp, 4], BF16, tag="E4")
                for j, (src, p0) in enumerate([(alf0, 0), (alm0, 0),
                                               (alm0, 3), (alf0, 3)]):
                    nc.scalar.activation(
                        bass.AP(tensor=E4.tensor, offset=E4.offset + j,
                                ap=[E4.ap[0], [4, rpp]]),
                        bass.AP(tensor=src.tensor, offset=src.offset + p0 * rpp,
                                ap=[src.ap[0], [1, rpp]]),
                        AF.Identity,
                    )

                if parts == "p1":
                    continue
                accs = []
                # ---- phase 2: mixing, pick, BCE (chunked) ----------------
                # PT lives in a column-interleaved layout: slot s holds
                # column PI[s], PI = [0,2,4,6,1,3,5,7]. Then every layer's
                # A-columns and B-columns are CONTIGUOUS flat slices:
                #   even layers: A = slots 0..3 (off 0),  B = slots 4..7 (off 32)
                #   odd layers:  A = slots 4..6 (off 32), B = slots 1..3 (off 8)
                # so d/a'/b' are 3D stt ops in the 4x DVE mode. The GT
                # constant (iota_pi) absorbs PI on the host side.
                for k in range(n_chunks):
                    pt_a = pp.tile([P, F, N * N], BF16, tag="pt_a")
                    pt_b = pp.tile([P, F, N * N], BF16, tag="pt_b")
                    # margin zeros (Pool): only positions read before written
                    for pt, offs in (
                        (pt_a, [(2, [10, 3], [1, 2]), (14, None, [1, 2]),
                                (40, [10, 3], [1, 2]), (48, None, [1, 2])]),
                        (pt_b, [(4, [32, 2], [1, 4]), (24, [32, 2], [1, 4]),
                                (16, None, [1, 2]), (46, None, [1, 2])]),
                    ):
                        for off, mid, inner in offs:
                            ap = [pt.ap[0], [N * N, F]]
                            if mid is not None:
                                ap.append(mid)
                            ap.append(inner)
                            nc.gpsimd.memset(
                                bass.AP(tensor=pt.tensor,
                                        offset=pt.offset + off, ap=ap), 0.0)

                    # analytic P2 (layers 0+1) into pt_a:
                    #   slots 4,5,6 (cols 1,3,5) <- A3 * B6[0:4]
                    #   slots 1,2,3 (cols 2,4,6) <- A3 * B6[2:6]
                    #   edges: col0 rows 0,1 / col7 rows 6,7 <- E4
                    A3k = bass.AP(tensor=A3.tensor, offset=A3.offset + k * F * 12,
                                  ap=[A3.ap[0], [12, F], [4, 3], [1, 4]])
                    for j0, out_off in ((0, 32), (2, 8)):
                        B6k = bass.AP(tensor=B6.tensor,
                                      offset=B6.offset + k * F * 18 + j0,
                                      ap=[B6.ap[0], [18, F], [6, 3], [1, 4]])
                        out = bass.AP(tensor=pt_a.tensor,
                                      offset=pt_a.offset + out_off,
                                      ap=[pt_a.ap[0], [N * N, F], [10, 3],
                                          [1, 4]])
                        nc.vector.tensor_tensor(out, A3k, B6k, A.mult)
                    nc.vector.tensor_copy(
                        bass.AP(tensor=pt_a.tensor, offset=pt_a.offset,
                                ap=[pt_a.ap[0], [N * N, F], [62, 2], [1, 2]]),
                        bass.AP(tensor=E4.tensor, offset=E4.offset + k * F * 4,
                                ap=[E4.ap[0], [4, F], [2, 2], [1, 2]]),
                    )

                    def flat_ap(pt, off, n):
                        return bass.AP(
                            tensor=pt.tensor, offset=pt.offset + off,
                            ap=[pt.ap[0], [N * N, F], [1, n]],
                        )

                    # tight A-side write groups per layer: (a_off, stride, n, w)
                    LGROUPS = {
                        2: [(0, 8, 1, 4), (8, 10, 2, 6), (28, 8, 1, 4)],
                        3: [(32, 8, 1, 6), (40, 8, 1, 8), (50, 8, 1, 6)],
                        4: [(0, 8, 1, 6), (8, 8, 2, 8), (26, 8, 1, 6)],
                        5: [(32, 8, 3, 8)],
                        6: [(0, 8, 4, 8)],
                        7: [(32, 8, 3, 8)],
                    }
                    pt_cur, pt_nxt = pt_a, pt_b
                    for layer in range(2, N):
                        st = layer % 2
                        npair = (N - st) // 2
                        wd = npair * N
                        a_off0 = 0 if st == 0 else 32
                        bdel = 32 if st == 0 else -24
                        al2k = al2s[layer][:, k * F:(k + 1) * F]
                        A_ap = flat_ap(pt_cur, a_off0, wd)
                        B_ap = flat_ap(pt_cur, a_off0 + bdel, wd)
                        d = mt.tile([P, F, wd], BF16, tag="d")
                        t = mt.tile([P, F, wd], BF16, tag="t")
                        al2_v = bass.AP(
                            tensor=al2k.tensor, offset=al2k.offset,
                            ap=[al2k.ap[0], [2, F * npair], [0, 4], [1, 2]],
                        )
                        d_v = bass.AP(
                            tensor=d.tensor, offset=d.offset,
                            ap=[d.ap[0], [N, F * npair], [2, 4], [1, 2]],
                        )
                        t_v2 = bass.AP(
                            tensor=t.tensor, offset=t.offset,
                            ap=[t.ap[0], [N, F * npair], [2, 4], [1, 2]],
                        )
                        nc.vector.tensor_tensor(d, A_ap, B_ap, A.subtract)
                        nc.vector.tensor_tensor(t_v2, al2_v, d_v, A.mult)
                        for (aoff, gstride, ng, w) in LGROUPS[layer]:
                            toff = aoff - a_off0
                            def wap(pt, off):
                                return bass.AP(
                                    tensor=pt.tensor, offset=pt.offset + off,
                                    ap=[pt.ap[0], [N * N, F], [gstride, ng],
                                        [1, w]])
                            tap = bass.AP(
                                tensor=t.tensor, offset=t.offset + toff,
                                ap=[t.ap[0], [wd, F], [gstride, ng], [1, w]])
                            dap_a = bass.AP(
                                tensor=pt_cur.tensor,
                                offset=pt_cur.offset + aoff,
                                ap=[pt_cur.ap[0], [N * N, F], [gstride, ng],
                                    [1, w]])
                            dap_b = bass.AP(
                                tensor=pt_cur.tensor,
                                offset=pt_cur.offset + aoff + bdel,
                                ap=[pt_cur.ap[0], [N * N, F], [gstride, ng],
                                    [1, w]])
                            nc.vector.tensor_tensor(
                                wap(pt_nxt, aoff), dap_b, tap, A.add)
                            nc.vector.tensor_tensor(
                                wap(pt_nxt, aoff + bdel), dap_a, tap,
                                A.subtract)
                        if st == 1:
                            # passthrough slots 0 (col 0) and 7 (col 7)
                            nc.vector.tensor_copy(
                                bass.AP(tensor=pt_nxt.tensor,
                                        offset=pt_nxt.offset,
                                        ap=[pt_nxt.ap[0], [N * N, F],
                                            [56, 2], [1, N]]),
                                bass.AP(tensor=pt_cur.tensor,
                                        offset=pt_cur.offset,
                                        ap=[pt_cur.ap[0], [N * N, F],
                                            [56, 2], [1, N]]),
                            )
                        pt_cur, pt_nxt = pt_nxt, pt_cur

                    # ---- GT one-hot pick + BCE ---------------------------
                    rank_k = rank_t[:, k * F:(k + 1) * F, :]
                    GT = rk.tile([P, F, N * N], BF16, tag="GT")
                    in_rank = bass.AP(
                        tensor=rank_t.tensor,
                        offset=rank_t.offset + k * F * N,
                        ap=[rank_t.ap[0], [N, F], [0, N], [1, N]],
                    )
                    in_iota = bass.AP(
                        tensor=iota_t.tensor, offset=iota_t.offset,
                        ap=[iota_t.ap[0], [0, F], [N, N], [1, N]],
                    )
                    GT4 = bass.AP(
                        tensor=GT.tensor, offset=GT.offset,
                        ap=[GT.ap[0], [N * N, F], [N, N], [1, N]],
                    )
                    nc.vector.tensor_tensor(GT4, in_rank, in_iota, A.is_equal)
                    # Q = GT * PT in place (flat 64)
                    tt(GT, GT, pt_cur, A.mult)
                    qs1 = rk.tile([P, F, N * N // 2], BF16, tag="qs1")
                    tt(qs1, GT[:, :, 0:32], GT[:, :, 32:64], A.add)
                    qs2 = rk.tile([P, F, N * N // 4], BF16, tag="qs2")
                    tt(qs2, qs1[:, :, 0:16], qs1[:, :, 16:32], A.add)
                    sel = rk.tile([P, F, N], BF16, tag="sel")
                    tt(sel, qs2[:, :, 0:8], qs2[:, :, 8:16], A.add)

                    acc1 = accp.tile([P, 1], F32, tag="acc1")
                    ln_scr = pp.tile([P, F, N * N], BF16, tag="ln_scr")
                    nc.scalar.activation(
                        ln_scr.rearrange("p a b -> p (a b)"),
                        pt_cur.rearrange("p a b -> p (a b)"),
                        AF.Ln, scale=-1.0, bias=1.0, accum_out=acc1,
                    )
                    acc2 = accp.tile([P, 1], F32, tag="acc2")
                    sel_scr = vt.tile([P, F, N], F16, tag="sel_scr")
                    nc.scalar.activation(
                        sel_scr.rearrange("p a b -> p (a b)"),
                        sel.rearrange("p a b -> p (a b)"),
                        AF.Ln, accum_out=acc2,
                    )
                    acc3 = accp.tile([P, 1], F32, tag="acc3")
                    sel_scr2 = vt.tile([P, F, N], F16, tag="sel_scr")
                    nc.scalar.activation(
                        sel_scr2.rearrange("p a b -> p (a b)"),
                        sel.rearrange("p a b -> p (a b)"),
                        AF.Ln, scale=-1.0, bias=1.0, accum_out=acc3,
                    )
                    accs.append((acc1, acc2, acc3))

                # combine all chunks' accumulators at pass end so the DVE
                # stream never waits on ACT accumulators mid-pass
                for a1, a2, a3 in accs:
                    nc.vector.tensor_tensor(a1, a1, a2, A.add)
                    nc.vector.tensor_tensor(a1, a1, a3, A.subtract)
                    nc.vector.tensor_tensor(total_t, total_t, a1, A.add)

            nc.gpsimd.dma_start(out=out_h[:], in_=total_t)

    nc.compile()
    return nc


_NC_CACHE = {}


def _get_nc(rows_per_core, chunk_rows=64, mix_bf16=True, repeats=1, parts="all"):
    key = (rows_per_core, chunk_rows, mix_bf16, repeats, parts)
    if key not in _NC_CACHE:
        _NC_CACHE[key] = build_nc(rows_per_core, chunk_rows, mix_bf16, repeats, parts)
    return _NC_CACHE[key]


PI = np.array([0, 2, 4, 6, 1, 3, 5, 7])  # column stored in PT slot s


def _iota_const(mix_bf16=True):
    # iota_pi[p, s*8 + r] = PI[s] -- GT one-hot targets in pi-layout;
    # tail 8 entries are plain 0..7 (rank init).
    row = np.concatenate([np.repeat(PI, N), np.arange(N)]).astype(ml_dtypes.bfloat16)
    return np.ascontiguousarray(np.broadcast_to(row, (P, N * N + N)))


def run_on_device(pred, labels, chunk_rows=64, mix_bf16=True, trace=False):
    """pred/labels: full [BATCH, 8] f32 (already ema-shifted). Returns
    (loss_scalar_f32, BassKernelResults)."""
    rows = pred.shape[0] // N_CORES
    nc = _get_nc(rows, chunk_rows, mix_bf16)
    iota = _iota_const(mix_bf16)
    in_maps = [
        {
            "pred": np.ascontiguousarray(pred[i * rows:(i + 1) * rows]),
            "labels": np.ascontiguousarray(labels[i * rows:(i + 1) * rows]),
            "iota_cr": iota,
        }
        for i in range(N_CORES)
    ]
    res = run_bass_kernel_spmd(nc, in_maps, list(range(N_CORES)), trace=trace)
    total = np.float64(0.0)
    for r in res.results:
        total += np.asarray(r["out"], dtype=np.float64).sum()
    loss = -total / (pred.shape[0] * N * N)
    return np.float32(loss), res


def kernel(pred_scores, labels, rank_ema):
    pred = np.asarray(pred_scores, dtype=np.float32)
    lab = np.asarray(labels, dtype=np.float32)
    ema = np.asarray(rank_ema, dtype=np.float32)
    if np.any(ema != 0.0):
        # General path: fold the (tiny) EMA shift on host; the device graph
        # is unchanged. rank_true = rank of each label.
        order = np.argsort(-lab, axis=-1, kind="stable")
        rank_true = np.argsort(order, axis=-1, kind="stable")
        pred = (pred - ema[rank_true]).astype(np.float32)
    loss, _ = run_on_device(pred, lab)
    return np.array(loss, dtype=np.float32)


# revision 25
# speedup vs baseline: 2.6288x; 2.6288x over previous
"""Trainium2 Bass kernel for AdDiffSortLoss (v2).

Reference computation (per batch row, n=8):
  rank_r      = # { j : labels[j] > labels[r] }          (descending rank)
  G[r, c]     = (rank_r == c)                            (one-hot GT permutation^T)
  x           = -(pred - rank_ema[rank])                 (rank_ema == 0 in practice)
  P           = odd-even differentiable sort network on x (8 layers, Cauchy CDF)
  loss        = -mean( G*clip(log P,-100) + (1-G)*clip(log1p(-P),-100) )

Loss decomposition on device (clips never bind: P in [1.9e-10, 0.975]):
  sum = SUM_all ln(1-P) + SUM_r [ ln(P[r,rank_r]) - ln(1-P[r,rank_r]) ]
  loss = -sum / (B*64)

Key implementation facts (hardware-verified this session):
  * The ACT Arctan table is accurate over the FULL input range (err <= 3e-7
    f32-out; f16-out limited only by f16 rounding), so alpha =
    0.5 + arctan(d)/pi needs NO range reduction: one table lookup per
    comparator, with the affine (1/pi, +0.5) folded into a second ACT
    Identity op (scale+bias). This deletes the |d|/reciprocal/is_ge/sign
    pipeline of the previous version.
  * On real HW, tensor_tensor with 2-byte dtypes and stride-1 innermost APs
    runs at ~0.475 ns/elem (2x); InstTensorScalarPtr runs at 1x (~1.1)
    DESPITE the cost model advertising 4x for it. All bulk math is therefore
    tensor_tensor.
  * Values run in f16 (|x| up to ~50 needs ~1e-3 abs resolution; bf16 is
    too coarse), the permutation mixing in bf16 (P entries down to 1.9e-10
    underflow f16). Ranks from bf16 labels (flips average out). End-to-end
    rel-err vs the f64 reference: 3.2e-04 (gate: 2e-2).
  * Layers 0+1 of the mixing are constructed ANALYTICALLY from the layer-0/1
    alphas (every P2 entry is a 2-term alpha product), which also removes
    the full ping-pong buffer zero-fill: only the 36 flat positions that
    later layers read-before-write are zeroed (small Pool memsets).
    Later layers write only their true column supports (tight windows).
  * Rank is computed with 7 shifted stride-1 compares + ragged in-place
    accumulates instead of an 8x8 broadcast compare matrix (broadcast
    operands break the 2x DVE mode).
  * The BCE ln scratch must NOT alias the idle ping-pong buffer: that WAW
    chained each chunk's ACT work into the next chunk's buffer reuse and
    cost ~80us/pass in stalls.

Engine split per pass: DVE does deltas/value updates/mixing/rank/pick;
ACT does arctan + alpha assembly + dtype casts + the P2 ingredient builds
and all BCE logs (with accumulate); Pool (GPSIMD) does the margin zeros.
Phase 2 runs in 32-row chunks with double-buffered PT tiles so chunk k+1
overlaps chunk k; per-chunk BCE accumulators are combined once at pass end
so the DVE stream never waits on the ACT accumulators mid-pass.

Sharding: pure data parallel over the batch across 8 NeuronCores; each core
reduces its shard to a [128,1] per-partition partial that the host sums.
"""

import math
import numpy as np

import concourse.bass as bass
import concourse.bacc as bacc
import concourse.tile as tile
from concourse import mybir
from concourse.bass_utils import run_bass_kernel_spmd

import ml_dtypes

F32 = mybir.dt.float32
F16 = mybir.dt.float16
BF16 = mybir.dt.bfloat16

N = 8                  # row width
N_CORES = 8
BATCH = 262144
CHUNK_ROWS = 32
ROWS_PER_CORE = BATCH // N_CORES   # 32768
P = 128                # partitions
RPP = ROWS_PER_CORE // P           # rows per partition (256)

A = mybir.AluOpType
AF = mybir.ActivationFunctionType
INV_PI = 1.0 / math.pi


def build_nc(rows_per_core=ROWS_PER_CORE, chunk_rows=32, mix_bf16=True,
             repeats=1, parts="all"):
    """Build the single-core SPMD Bass graph.

    chunk_rows: rows-per-partition per phase-2 chunk (must divide rows/128).
    repeats: process the whole shard this many times (timing builds only).
    """
    rpp = rows_per_core // P
    assert rpp * P == rows_per_core
    F = chunk_rows
    n_chunks = rpp // F
    assert n_chunks * F == rpp

    nc = bacc.Bacc("TRN2")

    pred_h = nc.declare_dram_parameter("pred", [rows_per_core, N], F32, isOutput=False)
    lab_h = nc.declare_dram_parameter("labels", [rows_per_core, N], F32, isOutput=False)
    # iota_cr[p, c*8+r] = c (replicated across partitions) -- for GT construction
    iota_h = nc.declare_dram_parameter("iota_cr", [P, N * N + N], BF16, isOutput=False)
    out_h = nc.declare_dram_parameter("out", [P, 1], F32, isOutput=True)

    predv = pred_h[:].rearrange("(p f) n -> p f n", p=P)   # [128, rpp, 8]
    labv = lab_h[:].rearrange("(p f) n -> p f n", p=P)

    def tt(out, in0, in1, op, engine=None):
        (engine or nc.vector).tensor_tensor(out, in0, in1, op)

    with tile.TileContext(nc) as tc:
        with (
            tc.tile_pool(name="io", bufs=2) as io,
            tc.tile_pool(name="rk", bufs=1) as rk,
            tc.tile_pool(name="pp", bufs=2 if chunk_rows <= 64 else 1) as pp,
            tc.tile_pool(name="vt", bufs=1) as vt,
            tc.tile_pool(name="als", bufs=1) as als,
            tc.tile_pool(name="mt", bufs=1) as mt,
            tc.tile_pool(name="acc", bufs=max(2, n_chunks)) as accp,
            tc.tile_pool(name="singles", bufs=1) as singles,
        ):
            # constants
            iota_t = singles.tile([P, N * N + N], BF16, tag="iota")
            nc.sync.dma_start(out=iota_t, in_=iota_h[:])
            # iota_r[p, r] = r lives in the constant's 8-wide tail
            iota_r = iota_t[:, N * N:N * N + N]
            half_t = singles.tile([P, 1], F32, tag="half")
            nc.vector.memset(half_t, 0.5)
            one_t = singles.tile([P, 1], F32, tag="one")
            nc.vector.memset(one_t, 1.0)
            total_t = singles.tile([P, 1], F32, tag="total")
            nc.vector.tensor_scalar(
                total_t, iota_t[:, 0:1], 0.0, None, A.mult
            )

            for _ in range(repeats):
                # ---- input loads: 4 DMA queues per tensor ----------------
                pred_t = io.tile([P, rpp, N], F32, tag="pred")
                lab_t = io.tile([P, rpp, N], F32, tag="lab")
                q = rpp // 4
                for i in range(4):
                    nc.sync.dma_start(out=pred_t[:, i * q:(i + 1) * q, :],
                                      in_=predv[:, i * q:(i + 1) * q, :])
                    nc.sync.dma_start(out=lab_t[:, i * q:(i + 1) * q, :],
                                      in_=labv[:, i * q:(i + 1) * q, :])

                # ---- casts on ACT ----------------------------------------
                # x slot-major [P, slot, f] f16; x = -10*pred
                x_a = vt.tile([P, N, rpp], F16, tag="x_a")
                x_b = vt.tile([P, N, rpp], F16, tag="x_b")
                nc.scalar.activation(
                    x_a,
                    bass.AP(tensor=pred_t.tensor, offset=pred_t.offset,
                            ap=[pred_t.ap[0], [1, N], [N, rpp]]),
                    AF.Identity, scale=-10.0,
                )
                labq = rk.tile([P, rpp, N], BF16, tag="labq")
                nc.scalar.activation(labq, lab_t, AF.Identity)

                # ---- rank via shifted comparisons (full width, bf16) -----
                rank_t = rk.tile([P, rpp, N], BF16, tag="rank")
                nc.vector.tensor_copy(
                    rank_t,
                    bass.AP(tensor=iota_r.tensor, offset=iota_r.offset,
                            ap=[iota_r.ap[0], [0, rpp], [1, N]]),
                )
                cs = rk.tile([P, rpp, N - 1], BF16, tag="cs")

                def rank_step(s):
                    # rank starts at r; j>r adds c_s, j<r adds 1-c_s whose +1
                    # is pre-folded into the iota init, leaving -c_s.
                    w = N - s
                    c_s = cs[:, :, 0:w]
                    tt(c_s, labq[:, :, s:N], labq[:, :, 0:N - s], A.is_gt)
                    tt(rank_t[:, :, 0:w], rank_t[:, :, 0:w], c_s, A.add)
                    tt(rank_t[:, :, s:N], rank_t[:, :, s:N], c_s, A.subtract)

                # ---- phase 1: value recurrence + alphas ------------------
                al2s = []
                alfs = []
                x_cur, x_nxt = x_a, x_b
                for layer in range(N):
                    st = layer % 2
                    npair = (N - st) // 2
                    # slot-major slices: [pair, f], innermost f stride 1
                    def slot_ap(x, base):
                        return bass.AP(
                            tensor=x.tensor, offset=x.offset + base * rpp,
                            ap=[x.ap[0], [2 * rpp, npair], [1, rpp]],
                        )
                    a_ap = slot_ap(x_cur, st)
                    b_ap = slot_ap(x_cur, st + 1)

                    delta = vt.tile([P, npair, rpp], F16, tag="delta")
                    tt(delta, b_ap, a_ap, A.subtract)
                    at16 = vt.tile([P, npair, rpp], F16, tag="at16")
                    nc.scalar.activation(at16, delta, AF.Arctan)
                    # alpha = at/pi + 0.5 assembled ON ACT (scale+bias).
                    # Only this op is on the recurrence critical path; the
                    # mixing-alpha dups and 1-alpha ops are deferred below
                    # so the in-order ACT queue stays on the recurrence.
                    alf = vt.tile([P, npair, rpp], F16, tag=f"alf_{layer}")
                    nc.scalar.activation(alf, at16, AF.Identity,
                                         scale=INV_PI, bias=half_t)
                    alfs.append(alf)
                    if layer < N - 1:
                        # fill the DVE bubble (waiting on ACT) with one
                        # independent rank step
                        rank_step(layer + 1)

                    if layer < N - 1:
                        tv = vt.tile([P, npair, rpp], F16, tag="tv")
                        tt(tv, alf, delta, A.mult)
                        tt(slot_ap(x_nxt, st), b_ap, tv, A.subtract)
                        tt(slot_ap(x_nxt, st + 1), a_ap, tv, A.add)
                        if st == 1:  # passthrough slots 0 and 7
                            nc.vector.tensor_copy(
                                bass.AP(tensor=x_nxt.tensor, offset=x_nxt.offset,
                                        ap=[x_nxt.ap[0], [(N - 1) * rpp, 2], [1, rpp]]),
                                bass.AP(tensor=x_cur.tensor, offset=x_cur.offset,
                                        ap=[x_cur.ap[0], [(N - 1) * rpp, 2], [1, rpp]]),
                            )
                        x_cur, x_nxt = x_nxt, x_cur

                # ---- deferred ACT work (off the recurrence path) ---------
                alfs01 = []
                for layer in (0, 1):
                    npair_l = (N - layer % 2) // 2
                    alm = vt.tile([P, npair_l, rpp], F16, tag=f"alm_{layer}")
                    nc.scalar.activation(alm, alfs[layer], AF.Identity,
                                         scale=-1.0, bias=one_t)
                    alfs01.append((alfs[layer], alm))
                    al2s.append(None)
                for layer in range(2, N):
                    npair_l = (N - layer % 2) // 2
                    alf = alfs[layer]
                    al2 = als.tile([P, rpp, npair_l, 2], BF16,
                                   tag=f"al2_{layer}")
                    nc.scalar.activation(
                        al2,
                        bass.AP(tensor=alf.tensor, offset=alf.offset,
                                ap=[alf.ap[0], [1, rpp], [rpp, npair_l],
                                    [0, 2]]),
                        AF.Identity,
                    )
                    al2s.append(al2)

                # ---- analytic P2 ingredients (bf16, full width) ----------
                # A3[f,k,:] = [1-a0_k, a0_k, a0_{k+1}, 1-a0_{k+1}]
                # B6[f,k,:] = [b_k, b_k, 1-b_k, 1-b_k, b_k, b_k]
                # E4[f,:]   = [a0_0, 1-a0_0, 1-a0_3, a0_3]
                (alf0, alm0), (alf1, alm1) = alfs01
                A3 = rk.tile([P, rpp, 3, 4], BF16, tag="A3")
                for j, (src, p0) in enumerate([(alm0, 0), (alf0, 0),
                                               (alf0, 1), (alm0, 1)]):
                    nc.scalar.activation(
                        bass.AP(tensor=A3.tensor, offset=A3.offset + j,
                                ap=[A3.ap[0], [12, rpp], [4, 3]]),
                        bass.AP(tensor=src.tensor, offset=src.offset + p0 * rpp,
                                ap=[src.ap[0], [1, rpp], [rpp, 3]]),
                        AF.Identity,
                    )
                B6 = rk.tile([P, rpp, 3, 6], BF16, tag="B6")
                for j0, src in [(0, alf1), (2, alm1), (4, alf1)]:
                    nc.scalar.activation(
                        bass.AP(tensor=B6.tensor, offset=B6.offset + j0,
                                ap=[B6.ap[0], [18, rpp], [6, 3], [1, 2]]),
                        bass.AP(tensor=src.tensor, offset=src.offset,
                                ap=[src.ap[0], [1, rpp], [rpp, 3], [0, 2]]),
                        AF.Identity,
                    )
                E4 = rk.tile([P, rpp, 4], BF16, tag="E4")
                for j, (src, p0) in enumerate([(alf0, 0), (alm0, 0),
                                               (alm0, 3), (alf0, 3)]):
                    nc.scalar.activation(
                        bass.AP(tensor=E4.tensor, offset=E4.offset + j,
                                ap=[E4.ap[0], [4, rpp]]),
                        bass.AP(tensor=src.tensor, offset=src.offset + p0 * rpp,
                                ap=[src.ap[0], [1, rpp]]),
                        AF.Identity,
                    )

                if parts == "p1":
                    continue
                accs = []
                # ---- phase 2: mixing, pick, BCE (chunked) ----------------
                # PT lives in a column-interleaved layout: slot s holds
                # column PI[s], PI = [0,2,4,6,1,3,5,7]. Then every layer's
                # A-columns and B-columns are CONTIGUOUS flat slices:
                #   even layers: A = slots 0..3 (off 0),  B = slots 4..7 (off 32)
                #   odd layers:  A = slots 4..6 (off 32), B = slots 1..3 (off 8)
                # so d/a'/b' are 3D stt ops in the 4x DVE mode. The GT
                # constant (iota_pi) absorbs PI on the host side.
                for k in range(n_chunks):
                    pt_a = pp.tile([P, F, N * N], BF16, tag="pt_a")
                    pt_b = pp.tile([P, F, N * N], BF16, tag="pt_b")
                    # margin zeros (Pool): only positions read before written
                    for pt, offs in (
                        (pt_a, [(2, [10, 3], [1, 2]), (14, None, [1, 2]),
                                (40, [10, 3], [1, 2]), (48, None, [1, 2])]),
                        (pt_b, [(4, [32, 2], [1, 4]), (24, [32, 2], [1, 4]),
                                (16, None, [1, 2]), (46, None, [1, 2])]),
                    ):
                        for off, mid, inner in offs:
                            ap = [pt.ap[0], [N * N, F]]
                            if mid is not None:
                                ap.append(mid)
                            ap.append(inner)
                            nc.gpsimd.memset(
                                bass.AP(tensor=pt.tensor,
                                        offset=pt.offset + off, ap=ap), 0.0)

                    # analytic P2 (layers 0+1) into pt_a:
                    #   slots 4,5,6 (cols 1,3,5) <- A3 * B6[0:4]
                    #   slots 1,2,3 (cols 2,4,6) <- A3 * B6[2:6]
                    #   edges: col0 rows 0,1 / col7 rows 6,7 <- E4
                    A3k = bass.AP(tensor=A3.tensor, offset=A3.offset + k * F * 12,
                                  ap=[A3.ap[0], [12, F], [4, 3], [1, 4]])
                    for j0, out_off in ((0, 32), (2, 8)):
                        B6k = bass.AP(tensor=B6.tensor,
                                      offset=B6.offset + k * F * 18 + j0,
                                      ap=[B6.ap[0], [18, F], [6, 3], [1, 4]])
                        out = bass.AP(tensor=pt_a.tensor,
                                      offset=pt_a.offset + out_off,
                                      ap=[pt_a.ap[0], [N * N, F], [10, 3],
                                          [1, 4]])
                        nc.vector.tensor_tensor(out, A3k, B6k, A.mult)
                    nc.vector.tensor_copy(
                        bass.AP(tensor=pt_a.tensor, offset=pt_a.offset,
                                ap=[pt_a.ap[0], [N * N, F], [62, 2], [1, 2]]),
                        bass.AP(tensor=E4.tensor, offset=E4.offset + k * F * 4,
                                ap=[E4.ap[0], [4, F], [2, 2], [1, 2]]),
                    )

                    def flat_ap(pt, off, n):
                        return bass.AP(
                            tensor=pt.tensor, offset=pt.offset + off,
                            ap=[pt.ap[0], [N * N, F], [1, n]],
                        )

                    # tight A-side write groups per layer: (a_off, stride, n, w)
                    LGROUPS = {
                        2: [(0, 8, 1, 4), (8, 10, 2, 6), (28, 8, 1, 4)],
                        3: [(32, 8, 1, 6), (40, 8, 1, 8), (50, 8, 1, 6)],
                        4: [(0, 8, 1, 6), (8, 8, 2, 8), (26, 8, 1, 6)],
                        5: [(32, 8, 3, 8)],
                        6: [(0, 8, 4, 8)],
                        7: [(32, 8, 3, 8)],
                    }
                    pt_cur, pt_nxt = pt_a, pt_b
                    for layer in range(2, N):
                        st = layer % 2
                        npair = (N - st) // 2
                        wd = npair * N
                        a_off0 = 0 if st == 0 else 32
                        bdel = 32 if st == 0 else -24
                        al2k = al2s[layer][:, k * F:(k + 1) * F]
                        A_ap = flat_ap(pt_cur, a_off0, wd)
                        B_ap = flat_ap(pt_cur, a_off0 + bdel, wd)
                        d = mt.tile([P, F, wd], BF16, tag="d")
                        t = mt.tile([P, F, wd], BF16, tag="t")
                        al2_v = bass.AP(
                            tensor=al2k.tensor, offset=al2k.offset,
                            ap=[al2k.ap[0], [2, F * npair], [0, 4], [1, 2]],
                        )
                        d_v = bass.AP(
                            tensor=d.tensor, offset=d.offset,
                            ap=[d.ap[0], [N, F * npair], [2, 4], [1, 2]],
                        )
                        t_v2 = bass.AP(
                            tensor=t.tensor, offset=t.offset,
                            ap=[t.ap[0], [N, F * npair], [2, 4], [1, 2]],
                        )
                        nc.vector.tensor_tensor(d, A_ap, B_ap, A.subtract)
                        nc.vector.tensor_tensor(t_v2, al2_v, d_v, A.mult)
                        for (aoff, gstride, ng, w) in LGROUPS[layer]:
                            toff = aoff - a_off0
                            def wap(pt, off):
                                return bass.AP(
                                    tensor=pt.tensor, offset=pt.offset + off,
                                    ap=[pt.ap[0], [N * N, F], [gstride, ng],
                                        [1, w]])
                            tap = bass.AP(
                                tensor=t.tensor, offset=t.offset + toff,
                                ap=[t.ap[0], [wd, F], [gstride, ng], [1, w]])
                            dap_a = bass.AP(
                                tensor=pt_cur.tensor,
                                offset=pt_cur.offset + aoff,
                                ap=[pt_cur.ap[0], [N * N, F], [gstride, ng],
                                    [1, w]])
                            dap_b = bass.AP(
                                tensor=pt_cur.tensor,
                                offset=pt_cur.offset + aoff + bdel,
                                ap=[pt_cur.ap[0], [N * N, F], [gstride, ng],
                                    [1, w]])
                            nc.vector.tensor_tensor(
                                wap(pt_nxt, aoff), dap_b, tap, A.add)
                            nc.vector.tensor_tensor(
                                wap(pt_nxt, aoff + bdel), dap_a, tap,
                                A.subtract)
                        if st == 1:
                            # passthrough slots 0 (col 0) and 7 (col 7)
                            nc.vector.tensor_copy(
                                bass.AP(tensor=pt_nxt.tensor,
                                        offset=pt_nxt.offset,
                                        ap=[pt_nxt.ap[0], [N * N, F],
                                            [56, 2], [1, N]]),
                                bass.AP(tensor=pt_cur.tensor,
                                        offset=pt_cur.offset,
                                        ap=[pt_cur.ap[0], [N * N, F],
                                            [56, 2], [1, N]]),
                            )
                        pt_cur, pt_nxt = pt_nxt, pt_cur

                    # ---- GT one-hot pick + BCE ---------------------------
                    rank_k = rank_t[:, k * F:(k + 1) * F, :]
                    GT = rk.tile([P, F, N * N], BF16, tag="GT")
                    in_rank = bass.AP(
                        tensor=rank_t.tensor,
                        offset=rank_t.offset + k * F * N,
                        ap=[rank_t.ap[0], [N, F], [0, N], [1, N]],
                    )
                    in_iota = bass.AP(
                        tensor=iota_t.tensor, offset=iota_t.offset,
                        ap=[iota_t.ap[0], [0, F], [N, N], [1, N]],
                    )
                    GT4 = bass.AP(
                        tensor=GT.tensor, offset=GT.offset,
                        ap=[GT.ap[0], [N * N, F], [N, N], [1, N]],
                    )
                    nc.vector.tensor_tensor(GT4, in_rank, in_iota, A.is_equal)
                    # Q = GT * PT in place (flat 64)
                    tt(GT, GT, pt_cur, A.mult)
                    qs1 = rk.tile([P, F, N * N // 2], BF16, tag="qs1")
                    tt(qs1, GT[:, :, 0:32], GT[:, :, 32:64], A.add)
                    qs2 = rk.tile([P, F, N * N // 4], BF16, tag="qs2")
                    tt(qs2, qs1[:, :, 0:16], qs1[:, :, 16:32], A.add)
                    sel = rk.tile([P, F, N], BF16, tag="sel")
                    tt(sel, qs2[:, :, 0:8], qs2[:, :, 8:16], A.add)

                    acc1 = accp.tile([P, 1], F32, tag="acc1")
                    ln_scr = pp.tile([P, F, N * N], BF16, tag="ln_scr")
                    nc.scalar.activation(
                        ln_scr.rearrange("p a b -> p (a b)"),
                        pt_cur.rearrange("p a b -> p (a b)"),
                        AF.Ln, scale=-1.0, bias=1.0, accum_out=acc1,
                    )
                    acc2 = accp.tile([P, 1], F32, tag="acc2")
                    sel_scr = vt.tile([P, F, N], F16, tag="sel_scr")
                    nc.scalar.activation(
                        sel_scr.rearrange("p a b -> p (a b)"),
                        sel.rearrange("p a b -> p (a b)"),
                        AF.Ln, accum_out=acc2,
                    )
                    acc3 = accp.tile([P, 1], F32, tag="acc3")
                    sel_scr2 = vt.tile([P, F, N], F16, tag="sel_scr")
                    nc.scalar.activation(
                        sel_scr2.rearrange("p a b -> p (a b)"),
                        sel.rearrange("p a b -> p (a b)"),
                        AF.Ln, scale=-1.0, bias=1.0, accum_out=acc3,
                    )
                    accs.append((acc1, acc2, acc3))

                # combine all chunks' accumulators at pass end so the DVE
                # stream never waits on ACT accumulators mid-pass
                for a1, a2, a3 in accs:
                    nc.vector.tensor_tensor(a1, a1, a2, A.add)
                    nc.vector.tensor_tensor(a1, a1, a3, A.subtract)
                    nc.vector.tensor_tensor(total_t, total_t, a1, A.add)

            nc.gpsimd.dma_start(out=out_h[:], in_=total_t)

    nc.compile()
    return nc


_NC_CACHE = {}


def _get_nc(rows_per_core, chunk_rows=32, mix_bf16=True, repeats=1, parts="all"):
    key = (rows_per_core, chunk_rows, mix_bf16, repeats, parts)
    if key not in _NC_CACHE:
        _NC_CACHE[key] = build_nc(rows_per_core, chunk_rows, mix_bf16, repeats, parts)
    return _NC_CACHE[key]


PI = np.array([0, 2, 4, 6, 1, 3, 5, 7])  # column stored in PT slot s


def _iota_const(mix_bf16=True):
    # iota_pi[p, s*8 + r] = PI[s] -- GT one-hot targets in pi-layout;
    # tail 8 entries are plain 0..7 (rank init).
    row = np.concatenate([np.repeat(PI, N), np.arange(N)]).astype(ml_dtypes.bfloat16)
    return np.ascontiguousarray(np.broadcast_to(row, (P, N * N + N)))


def run_on_device(pred, labels, chunk_rows=32, mix_bf16=True, trace=False):
    """pred/labels: full [BATCH, 8] f32 (already ema-shifted). Returns
    (loss_scalar_f32, BassKernelResults)."""
    rows = pred.shape[0] // N_CORES
    nc = _get_nc(rows, chunk_rows, mix_bf16)
    iota = _iota_const(mix_bf16)
    in_maps = [
        {
            "pred": np.ascontiguousarray(pred[i * rows:(i + 1) * rows]),
            "labels": np.ascontiguousarray(labels[i * rows:(i + 1) * rows]),
            "iota_cr": iota,
        }
        for i in range(N_CORES)
    ]
    res = run_bass_kernel_spmd(nc, in_maps, list(range(N_CORES)), trace=trace)
    total = np.float64(0.0)
    for r in res.results:
        total += np.asarray(r["out"], dtype=np.float64).sum()
    loss = -total / (pred.shape[0] * N * N)
    return np.float32(loss), res


def kernel(pred_scores, labels, rank_ema):
    pred = np.asarray(pred_scores, dtype=np.float32)
    lab = np.asarray(labels, dtype=np.float32)
    ema = np.asarray(rank_ema, dtype=np.float32)
    if np.any(ema != 0.0):
        # General path: fold the (tiny) EMA shift on host; the device graph
        # is unchanged. rank_true = rank of each label.
        order = np.argsort(-lab, axis=-1, kind="stable")
        rank_true = np.argsort(order, axis=-1, kind="stable")
        pred = (pred - ema[rank_true]).astype(np.float32)
    loss, _ = run_on_device(pred, lab)
    return np.array(loss, dtype=np.float32)
